# revision 22
# baseline (speedup 1.0000x reference)
"""BiLSTM tagger kernel, direction-parallel over 8 NeuronCores, v4.

Core c in 0..3 runs the FORWARD direction for sequences [32c, 32c+32);
core c+4 runs the BACKWARD direction for the same sequences. The backward
scan runs over GLOBALLY reversed time (host flips x along t); per-sequence
ragged reversal is handled by masking the i/f/o gates each step (mask=0
keeps h=c=0 through the pad region), which reproduces pad-packed semantics
exactly with no per-sequence index tensors.

Tokens are TIME-MAJOR (tok = t*32 + b) and all inter-phase tensors live in
TRANSPOSED layout hT[128, Tn*128] (col block t = 4 H-chunks x 32 batch),
written directly from the scan's per-step PE transpose. Backward cores
write their hT at column T-1-t, so every stored tensor is in forward time
order and the layer-2 projection + classifier read PLAIN contiguous
blocks: no indirect gathers and no DMA transposes anywhere. The backward
core's own layer-2 projection reads forward-time blocks as-is (inner
4-step order reversed) and its scan un-permutes by indexing the gx window
with 3-(t%4).

Scan: col-tiled quadrant psum layout - the 16 recurrent matmuls run as 4
rounds of 4 CONCURRENT col-tiled matmuls (tile_position=(0,32q)); psum
quadrant q = gate q (order i,f,o,g), sigmoid(i,f,o) is ONE [96,512] op.
All matmuls bf16, psum fp32.
"""

import sys

for _p in ("/opt/trn_rl_repo",):
    if _p not in sys.path:
        sys.path.append(_p)

import numpy as np
import ml_dtypes

import concourse.bass as bass
import concourse.tile as tile
from concourse import bacc, mybir
from concourse.bass import IndirectOffsetOnAxis
from concourse.bass_utils import run_bass_kernel_spmd

F32 = mybir.dt.float32
BF16 = mybir.dt.bfloat16
I32 = mybir.dt.int32
AF = mybir.ActivationFunctionType
ALU = mybir.AluOpType

B, T, V, E, H, TAGS = 128, 512, 50000, 256, 512, 64
NC = 8
NPAIR = NC // 2
BL = B // NPAIR          # 32 sequences per core (one direction each)
G = 4 * H
GROUPS = [[c, c + NPAIR] for c in range(NPAIR)]
NTOK = BL * T            # 16384 tokens per core, time-major: tok = t*32+b

# gate order i,f,o,g: psum quadrant q holds gate q, sigmoid covers 0:96
_GATE_PERM = np.concatenate([
    np.arange(0, H), np.arange(H, 2 * H), np.arange(3 * H, 4 * H),
    np.arange(2 * H, 3 * H)])


def _build(nc, Tn=T, Bl=BL):
    nchunk = NTOK // 128     # 128 token chunks (4 steps each)
    ncls = nchunk // 2       # 64 classifier chunks per core
    KE = E // 128            # 2
    KH = H // 128            # 4
    KH2 = 2 * H // 128       # 8

    emb = nc.dram_tensor("emb", [V, E], F32, kind="ExternalInput")
    xg_idx = nc.dram_tensor("xg_idx", [128, nchunk], I32, kind="ExternalInput")
    mask = nc.dram_tensor("mask", [Bl, Tn], F32, kind="ExternalInput")
    ident = nc.dram_tensor("ident", [32, 32], BF16, kind="ExternalInput")
    id128 = nc.dram_tensor("id128", [128, 128], BF16, kind="ExternalInput")
    flagF = nc.dram_tensor("flagF", [128, 1], F32, kind="ExternalInput")
    flagB = nc.dram_tensor("flagB", [128, 1], F32, kind="ExternalInput")

    wih, whh, biasd = {}, {}, {}
    for l, din in (("l1", E), ("l2", 2 * H)):
        wih[l] = nc.dram_tensor(f"wihT_{l}", [din, G], BF16, kind="ExternalInput")
        whh[l] = nc.dram_tensor(f"whhT_{l}", [H, G], BF16, kind="ExternalInput")
        biasd[l] = nc.dram_tensor(f"bias_{l}", [128, G], F32, kind="ExternalInput")
    wcls = nc.dram_tensor("wclsT", [2 * H, TAGS], BF16, kind="ExternalInput")
    bcls = nc.dram_tensor("bcls", [TAGS, 1], F32, kind="ExternalInput")

    gx = {l: nc.dram_tensor(f"gx_{l}", [NTOK, G], BF16) for l in ("l1", "l2")}
    hT = {l: nc.dram_tensor(f"hT_{l}", [128, Tn * 128], BF16)
          for l in ("l1", "l2")}
    hTp = {l: nc.dram_tensor(f"hTp_{l}", [2 * 128, Tn * 128], BF16)
           for l in ("l1", "l2")}
    logitsT = nc.dram_tensor("logitsT", [TAGS, NTOK // 2], F32,
                             kind="ExternalOutput")

    # Every core writes its hT in OWN-scan order (uniform). In the
    # exchanged buffer slot0 = fwd core's hT (columns = fwd time) and
    # slot1 = bwd core's (columns = bwd scan time = T-1-fwd). A core
    # reading x2 at its own scan step t needs the OWN slot natural and
    # the CROSS slot time-reversed; which slot is which depends on the
    # core, so both variants are loaded (cheap contiguous DMA) and
    # selected with a host 0/1 flag on the DVE (SPMD-uniform program).

    with tile.TileContext(nc) as tc:
        with tc.tile_pool(name="const", bufs=1) as cpool:
            def load_const(nm, shape, dt, src_ap):
                t = cpool.tile(shape, dt, name=nm, tag=nm)
                nc.gpsimd.dma_start(t[:], src_ap)
                return t

            xg_sb = load_const("xg_sb", [128, nchunk], I32, xg_idx[:])
            mask_sb = load_const("mask_sb", [Bl, Tn], F32, mask[:])
            id_sb = load_const("id_sb", [32, 32], BF16, ident[:])
            id128_sb = load_const("id128_sb", [128, 128], BF16, id128[:])
            bcls_sb = load_const("bcls_sb", [TAGS, 1], F32, bcls[:])
            fF_sb = load_const("fF_sb", [128, 1], F32, flagF[:])
            fB_sb = load_const("fB_sb", [128, 1], F32, flagB[:])
            bias_sb = {l: load_const(f"bias_sb_{l}", [128, G], F32, biasd[l][:])
                       for l in ("l1", "l2")}

            _proj1(nc, tc, nchunk, KE, wih["l1"], bias_sb["l1"], gx["l1"],
                   emb, xg_sb, id128_sb)
            _scan(nc, tc, Tn, Bl, KH, whh["l1"], gx["l1"], hT["l1"],
                  mask_sb, id_sb)
            nc.gpsimd.collective_compute(
                "AllGather", ALU.bypass, GROUPS,
                ins=[hT["l1"][:]], outs=[hTp["l1"][:]])
            _proj2(nc, tc, nchunk, Tn, wih["l2"], bias_sb["l2"], gx["l2"],
                   hTp["l1"], fF_sb, fB_sb)
            _scan(nc, tc, Tn, Bl, KH, whh["l2"], gx["l2"], hT["l2"],
                  mask_sb, id_sb)
            nc.gpsimd.collective_compute(
                "AllGather", ALU.bypass, GROUPS,
                ins=[hT["l2"][:]], outs=[hTp["l2"][:]])
            _classifier(nc, tc, ncls, Tn, wcls, bcls_sb, hTp["l2"], logitsT,
                        KH2, fF_sb, fB_sb)
    return nc


def _proj1(nc, tc, nchunk, KE, wih_d, bias_t, gx_d, emb, xg_sb, id128_sb):
    """gx1 = emb[x] @ W_ih1^T + b, time-major chunks of 128 tokens.
    Embedding rows gathered (fp32), cast, transposed on the PE."""
    with tc.tile_pool(name="pw", bufs=1) as wpool, \
         tc.tile_pool(name="pg", bufs=3) as gpool, \
         tc.tile_pool(name="pxps", bufs=2, space="PSUM") as xpspool, \
         tc.tile_pool(name="pps", bufs=4, space="PSUM") as ppool, \
         tc.tile_pool(name="pout", bufs=3) as opool:
        wsb = wpool.tile([128, KE, G], BF16, tag="w", name="wih1_sb")
        for k in range(KE):
            nc.gpsimd.dma_start(wsb[:, k, :], wih_d[128 * k:128 * (k + 1), :])
        for s in range(nchunk):
            e32 = gpool.tile([128, E], F32, tag="e32")
            nc.gpsimd.indirect_dma_start(
                out=e32[:], out_offset=None, in_=emb[:],
                in_offset=IndirectOffsetOnAxis(ap=xg_sb[:, s:s + 1], axis=0))
            e16 = gpool.tile([128, E], BF16, tag="e16")
            nc.vector.tensor_copy(e16[:], e32[:])
            xps = xpspool.tile([128, E], F32, tag="xps", name="xps")
            for kk in range(KE):
                nc.tensor.matmul(xps[:, 128 * kk:128 * (kk + 1)],
                                 e16[:, 128 * kk:128 * (kk + 1)], id128_sb[:],
                                 start=True, stop=True, skip_group_check=True)
            xT = gpool.tile([128, E], BF16, tag="xT")
            nc.scalar.activation(xT[:], xps[:], AF.Copy)
            gout = opool.tile([128, G], BF16, tag="gout")
            for n in range(4):
                ps = ppool.tile([128, 512], F32, tag="ps", name="pps")
                for kk in range(KE):
                    nc.tensor.matmul(
                        ps[:], xT[:, 128 * kk:128 * (kk + 1)],
                        wsb[:, kk, 512 * n:512 * (n + 1)],
                        start=(kk == 0), stop=(kk == KE - 1),
                        skip_group_check=True)
                nc.vector.tensor_tensor(
                    out=gout[:, 512 * n:512 * (n + 1)], in0=ps[:],
                    in1=bias_t[:, 512 * n:512 * (n + 1)], op=ALU.add)
            nc.gpsimd.dma_start(gx_d[128 * s:128 * (s + 1), :], gout[:])


def _load_x2(nc, xpool, hv, s, Tn, fF, fB, tag):
    """Load x2^T [128, 8, 4, 32] for chunk s (this core's scan steps
    4s..4s+4): slot d chunks k at rows 128d.. Both the natural and the
    time-reversed variant of each slot are loaded (contiguous DMA) and
    blended with the core's 0/1 flags: own slot natural, cross reversed.
    hv dims: [d, p, k, t, c32] (k before t so AP dim order matches dst)."""
    # tiles are t-major [128, d, t4, k4, c32] so each (slot, variant) is
    # ONE DMA: src AP dims (t, k, c) with strides (128, 32, 1); the
    # reversed variant just walks t backwards. Matmul lhsT for K-chunk kk
    # is the strided slice [:, d, :, kk&3, :].
    xn = xpool.tile([128, 2, 4, 4, 32], BF16, tag=tag + "n")
    xr = xpool.tile([128, 2, 4, 4, 32], BF16, tag=tag + "r")
    hi = Tn - 1 - 4 * s
    rsl = slice(hi, None, -1) if hi - 4 < 0 else slice(hi, hi - 4, -1)
    for d in range(2):
        nc.gpsimd.dma_start(xn[:, d, :, :, :], hv[d, :, 4 * s:4 * s + 4, :, :])
        nc.gpsimd.dma_start(xr[:, d, :, :, :], hv[d, :, rsl, :, :])
    # select per K-chunk so xT ends up K-MAJOR [p, d, k, t, c] with each
    # chunk's (t, c) contiguous - a legal one-free-dim matmul stationary.
    # slot0 (fwd dir): natural on fwd cores, reversed on bwd cores;
    # slot1 (bwd dir): the opposite.
    xT = xpool.tile([128, 2, 4, 4, 32], BF16, tag=tag)
    a = xpool.tile([128, 4, 32], BF16, tag=tag + "a")
    b = xpool.tile([128, 4, 32], BF16, tag=tag + "b")
    for d in range(2):
        fn, fr = (fF, fB) if d == 0 else (fB, fF)
        for k in range(4):
            nc.vector.tensor_scalar_mul(a[:], xn[:, d, :, k, :], fn[:, 0:1])
            nc.vector.tensor_scalar_mul(b[:], xr[:, d, :, k, :], fr[:, 0:1])
            nc.vector.tensor_tensor(out=xT[:, d, k, :, :], in0=a[:], in1=b[:],
                                    op=ALU.add)
    return xT


def _proj2(nc, tc, nchunk, Tn, wih_d, bias_t, gx_d, hTp_d, fF, fB):
    """gx2 = [out_f | out_b] @ W_ih2^T + b in this core's own scan order."""
    hv = hTp_d.ap().rearrange("(d p) (t k c) -> d p t k c", d=2, k=4, c=32)
    KD = 8
    with tc.tile_pool(name="qw", bufs=1) as wpool, \
         tc.tile_pool(name="qx", bufs=3) as xpool, \
         tc.tile_pool(name="qps", bufs=4, space="PSUM") as ppool, \
         tc.tile_pool(name="qout", bufs=3) as opool:
        wsb = wpool.tile([128, KD, G], BF16, tag="w", name="wih2_sb")
        for k in range(KD):
            nc.gpsimd.dma_start(wsb[:, k, :], wih_d[128 * k:128 * (k + 1), :])
        for s in range(nchunk):
            xT = _load_x2(nc, xpool, hv, s, Tn, fF, fB, "xT")
            gout = opool.tile([128, G], BF16, tag="gout")
            for n in range(4):
                ps = ppool.tile([128, 512], F32, tag="ps", name="qpps")
                for kk in range(KD):
                    nc.tensor.matmul(
                        ps[:], xT[:, kk // 4, kk % 4, :, :],
                        wsb[:, kk, 512 * n:512 * (n + 1)],
                        start=(kk == 0), stop=(kk == KD - 1),
                        skip_group_check=True)
                nc.vector.tensor_tensor(
                    out=gout[:, 512 * n:512 * (n + 1)], in0=ps[:],
                    in1=bias_t[:, 512 * n:512 * (n + 1)], op=ALU.add)
            nc.gpsimd.dma_start(gx_d[128 * s:128 * (s + 1), :], gout[:])


def _classifier(nc, tc, ncls, Tn, wcls_d, bcls_sb, hTp_d, logitsT, KH2,
                fF, fB):
    """logits for this core's half of the pair's tokens: chunks s=0..63 of
    its OWN scan time (host un-reverses bwd cores)."""
    hv = hTp_d.ap().rearrange("(d p) (t k c) -> d p t k c", d=2, k=4, c=32)
    with tc.tile_pool(name="cw", bufs=1) as wpool, \
         tc.tile_pool(name="cx", bufs=3) as xpool, \
         tc.tile_pool(name="cps", bufs=4, space="PSUM") as ppool, \
         tc.tile_pool(name="cout", bufs=3) as opool:
        wsb = wpool.tile([128, KH2, TAGS], BF16, tag="w", name="wcls_sb")
        for k in range(KH2):
            nc.gpsimd.dma_start(wsb[:, k, :], wcls_d[128 * k:128 * (k + 1), :])
        for s in range(ncls):
            o2T = _load_x2(nc, xpool, hv, s, Tn, fF, fB, "o2T")
            ps = ppool.tile([TAGS, 128], F32, tag="ps", name="cpps")
            for kk in range(KH2):
                nc.tensor.matmul(ps[:], wsb[:, kk, :],
                                 o2T[:, kk // 4, kk % 4, :, :],
                                 start=(kk == 0), stop=(kk == KH2 - 1),
                                 skip_group_check=True)
            lg = opool.tile([TAGS, 128], F32, tag="lg")
            nc.scalar.activation(lg[:], ps[:], AF.Identity,
                                 bias=bcls_sb[:, 0:1])
            nc.gpsimd.dma_start(logitsT[:, 128 * s:128 * (s + 1)], lg[:])


def _scan(nc, tc, Tn, Bl, KH, whh_d, gx_d, hTout_d, mask_sb, id_sb):
    """Single-direction scan, M=32, col-tiled quadrant psum layout.
    Gate masks (i,f,o multiplied by mask[:,t]) implement pad-packed
    semantics; the transposed state hTn is DMA'd per step straight into
    hTout (this core's scan order)."""
    TC = 4
    gxv = gx_d.ap().rearrange("(t b) d -> b t d", b=Bl)
    hTv = hTout_d.ap().rearrange("p (t c) -> p t c", c=128)
    with tc.tile_pool(name="sw", bufs=1) as wpool, \
         tc.tile_pool(name="sgx", bufs=3) as gxpool, \
         tc.tile_pool(name="sst", bufs=1) as stpool, \
         tc.tile_pool(name="sps", bufs=2, space="PSUM") as pspool, \
         tc.tile_pool(name="stps", bufs=2, space="PSUM") as tpspool, \
         tc.tile_pool(name="swk", bufs=3) as wkpool, \
         tc.tile_pool(name="shT", bufs=3) as htpool, \
         tc.tile_pool(name="srng", bufs=2) as rpool:
        wsb = wpool.tile([128, KH, G], BF16, tag="whh", name="whh_sb")
        for k in range(KH):
            nc.gpsimd.dma_start(wsb[:, k, :], whh_d[128 * k:128 * (k + 1), :])
        hT = [htpool.tile([128, KH * Bl], BF16, tag="hT", name="hT0")]
        nc.vector.memset(hT[0][:], 0.0)
        c_st = stpool.tile([Bl, H], BF16, tag="c", name="c_st")
        nc.vector.memset(c_st[:], 0.0)
        gxc = {}
        gps = [None]
        ring = [None]
        nwin = (Tn + TC - 1) // TC

        def load_gx(w):
            tl = gxpool.tile([Bl, TC, G], BF16, tag="gx", name="gxc")
            nc.gpsimd.dma_start(tl[:], gxv[:, w * TC:(w + 1) * TC, :])
            gxc[w] = tl
            gxc.pop(w - 2, None)

        def inject(tt):
            gps[0] = pspool.tile([128, H], F32, tag="ps", name="gps")
            gxt = gxc[tt // TC]
            j = tt % TC
            for q in range(4):
                nc.tensor.matmul(
                    gps[0][32 * q:32 * (q + 1), :], id_sb[:],
                    gxt[:, j, 512 * q:512 * (q + 1)],
                    start=True, stop=False, tile_position=(0, 32 * q),
                    skip_group_check=True)

        load_gx(0)
        if nwin > 1:
            load_gx(1)
        inject(0)
        for t in range(Tn):
            gc = gps[0]
            for k in range(KH):
                for q in range(4):
                    nc.tensor.matmul(
                        gc[32 * q:32 * (q + 1), :],
                        hT[0][:, Bl * k:Bl * (k + 1)],
                        wsb[:, k, 512 * q:512 * (q + 1)],
                        start=False, stop=(k == KH - 1),
                        tile_position=(0, 32 * q), skip_group_check=True)
            gact = wkpool.tile([128, H], BF16, tag="gact", name="gact")
            nc.scalar.activation(gact[0:96, :], gc[0:96, :], AF.Sigmoid)
            # i masked in place (base 0), f/o realigned+masked while tanh(g)
            # runs on ScalarE; mask=0 freezes h=c=0 (pad-packed semantics)
            gi0 = wkpool.tile([Bl, H], BF16, tag="gi0", name="gi0")
            nc.vector.tensor_scalar_mul(gi0[:], gact[0:32, :],
                                        mask_sb[:, t:t + 1])
            gf0 = wkpool.tile([Bl, H], BF16, tag="gf0", name="gf0")
            nc.vector.tensor_copy(gf0[:], gact[32:64, :])
            gf0m = wkpool.tile([Bl, H], BF16, tag="gf0m", name="gf0m")
            nc.vector.tensor_scalar_mul(gf0m[:], gf0[:], mask_sb[:, t:t + 1])
            gg0 = wkpool.tile([Bl, H], BF16, tag="gg0", name="gg0")
            nc.scalar.activation(gg0[:], gc[96:128, :], AF.Tanh)
            t1 = wkpool.tile([Bl, H], BF16, tag="t1", name="t1")
            nc.vector.tensor_tensor(out=t1[:], in0=gf0m[:], in1=c_st[:],
                                    op=ALU.mult)
            t2 = wkpool.tile([Bl, H], BF16, tag="t2", name="t2")
            nc.vector.tensor_tensor(out=t2[:], in0=gi0[:], in1=gg0[:],
                                    op=ALU.mult)
            nc.vector.tensor_tensor(out=c_st[:], in0=t1[:], in1=t2[:],
                                    op=ALU.add)
            tch = wkpool.tile([Bl, H], BF16, tag="tch", name="tch")
            nc.scalar.activation(tch[:], c_st[:], AF.Tanh)
            go0 = wkpool.tile([Bl, H], BF16, tag="go0", name="go0")
            nc.vector.tensor_copy(go0[:], gact[64:96, :])
            go0m = wkpool.tile([Bl, H], BF16, tag="go0m", name="go0m")
            nc.vector.tensor_scalar_mul(go0m[:], go0[:], mask_sb[:, t:t + 1])
            h16 = wkpool.tile([Bl, H], BF16, tag="h16", name="h16")
            nc.vector.tensor_tensor(out=h16[:], in0=go0m[:], in1=tch[:],
                                    op=ALU.mult)
            hT_ps = tpspool.tile([128, KH * Bl], F32, tag="tps", name="hT_ps")
            if t + 1 < Tn:
                if (t + 1) % TC == 0 and (t + 1) // TC + 1 < nwin:
                    load_gx((t + 1) // TC + 1)
                inject(t + 1)
            hTn = htpool.tile([128, KH * Bl], BF16, tag="hT", name="hTn")
            for k in range(KH):
                nc.tensor.matmul(hT_ps[:, Bl * k:Bl * (k + 1)],
                                 h16[:, 128 * k:128 * (k + 1)], id_sb[:],
                                 start=True, stop=True, skip_group_check=True)
            nc.scalar.activation(hTn[:], hT_ps[:], AF.Copy)
            hT[0] = hTn
            # ring-batch the hT writes: one [128, 4, 128] DMA per 4 steps
            if t % 4 == 0:
                ring[0] = rpool.tile([128, 4, 128], BF16, tag="rng",
                                     name="ring")
            nc.vector.tensor_copy(ring[0][:, t % 4, :], hTn[:])
            if (t + 1) % 4 == 0:
                nc.gpsimd.dma_start(hTv[:, t - 3:t + 1, :], ring[0][:])


def _prep_inputs(inputs, Tn=T, Bl=BL):
    x = np.asarray(inputs["x"]).astype(np.int32)
    lengths = np.asarray(inputs["lengths"]).astype(np.int32)
    emb = np.asarray(inputs["emb"], dtype=np.float32)
    bf = ml_dtypes.bfloat16

    wt = {}
    for s in ("f1", "b1", "f2", "b2"):
        w_ih = np.asarray(inputs[f"W_ih_{s}"], np.float32)[_GATE_PERM]
        w_hh = np.asarray(inputs[f"W_hh_{s}"], np.float32)[_GATE_PERM]
        b = np.asarray(inputs[f"b_{s}"], np.float32)[_GATE_PERM]
        wt[f"wihT_{s}"] = np.ascontiguousarray(w_ih.T).astype(bf)
        wt[f"whhT_{s}"] = np.ascontiguousarray(w_hh.T).astype(bf)
        wt[f"bias_{s}"] = np.tile(b.reshape(1, G), (128, 1))
    com = {"emb": emb, "ident": np.eye(32, dtype=bf),
           "id128": np.eye(128, dtype=bf),
           "wclsT": np.ascontiguousarray(
               np.asarray(inputs["W_cls"], np.float32).T).astype(bf),
           "bcls": np.asarray(inputs["b_cls"], np.float32).reshape(TAGS, 1)}

    def chunked_timemajor(xscan):
        # v[tok] = xscan[b, t] with tok = t*32 + b  ->  idx[p, s] = v[128s+p]
        v = np.ascontiguousarray(xscan.T).reshape(-1)   # [t, b] flat
        return np.ascontiguousarray(v.reshape(-1, 128).T).astype(np.int32)

    ts = np.arange(Tn)[None, :]
    in_maps = [None] * NC
    for p in range(NPAIR):
        xs = x[Bl * p:Bl * (p + 1), :Tn]
        ls = np.minimum(lengths[Bl * p:Bl * (p + 1)], Tn)[:, None]
        for half, core in ((0, p), (1, p + NPAIR)):
            if half == 0:   # forward
                xscan = xs
                m = (ts < ls).astype(np.float32)
                sfx = ("f1", "f2")
            else:           # backward: global time flip + tail mask
                xscan = xs[:, ::-1]
                m = (ts >= Tn - ls).astype(np.float32)
                sfx = ("b1", "b2")
            fl = 1.0 if half == 0 else 0.0
            im = {"xg_idx": chunked_timemajor(xscan), "mask": m,
                  "flagF": np.full((128, 1), fl, np.float32),
                  "flagB": np.full((128, 1), 1.0 - fl, np.float32),
                  "wihT_l1": wt[f"wihT_{sfx[0]}"],
                  "whhT_l1": wt[f"whhT_{sfx[0]}"],
                  "bias_l1": wt[f"bias_{sfx[0]}"],
                  "wihT_l2": wt[f"wihT_{sfx[1]}"],
                  "whhT_l2": wt[f"whhT_{sfx[1]}"],
                  "bias_l2": wt[f"bias_{sfx[1]}"]}
            im.update(com)
            in_maps[core] = im
    return in_maps


_CACHED = {}


def kernel(**inputs) -> np.ndarray:
    if "nc" not in _CACHED:
        nc = bacc.Bacc("TRN2", target_bir_lowering=False, debug=False,
                       num_devices=NC)
        _build(nc)
        nc.compile()
        _CACHED["nc"] = nc
    nc = _CACHED["nc"]
    in_maps = _prep_inputs(inputs)
    res = run_bass_kernel_spmd(nc, in_maps, core_ids=list(range(NC)),
                               trace=False)
    out = np.empty((B, T, TAGS), np.float32)
    half_T = T // 2
    for p in range(NPAIR):
        for half, core in ((0, p), (1, p + NPAIR)):
            lt = res.results[core]["logitsT"]          # [TAGS, 8192]
            seq = lt.T.reshape(half_T, BL, TAGS)       # [t_scan, b, TAGS]
            seq = np.transpose(seq, (1, 0, 2))         # [b, t_scan, TAGS]
            if half == 0:   # fwd core: scan time = fwd time 0..256
                out[BL * p:BL * (p + 1), 0:half_T] = seq
            else:           # bwd core: scan steps 0..256 = fwd time 511..256
                out[BL * p:BL * (p + 1), half_T:T] = seq[:, ::-1]
    return out.astype(np.float32)


# revision 23
# speedup vs baseline: 1.0109x; 1.0109x over previous
"""BiLSTM tagger kernel, direction-parallel over 8 NeuronCores, v4.

Core c in 0..3 runs the FORWARD direction for sequences [32c, 32c+32);
core c+4 runs the BACKWARD direction for the same sequences. The backward
scan runs over GLOBALLY reversed time (host flips x along t); per-sequence
ragged reversal is handled by masking the i/f/o gates each step (mask=0
keeps h=c=0 through the pad region), which reproduces pad-packed semantics
exactly with no per-sequence index tensors.

Tokens are TIME-MAJOR (tok = t*32 + b) and all inter-phase tensors live in
TRANSPOSED layout hT[128, Tn*128] (col block t = 4 H-chunks x 32 batch),
written directly from the scan's per-step PE transpose. Backward cores
write their hT at column T-1-t, so every stored tensor is in forward time
order and the layer-2 projection + classifier read PLAIN contiguous
blocks: no indirect gathers and no DMA transposes anywhere. The backward
core's own layer-2 projection reads forward-time blocks as-is (inner
4-step order reversed) and its scan un-permutes by indexing the gx window
with 3-(t%4).

Scan: col-tiled quadrant psum layout - the 16 recurrent matmuls run as 4
rounds of 4 CONCURRENT col-tiled matmuls (tile_position=(0,32q)); psum
quadrant q = gate q (order i,f,o,g), sigmoid(i,f,o) is ONE [96,512] op.
All matmuls bf16, psum fp32.
"""

import sys

for _p in ("/opt/trn_rl_repo",):
    if _p not in sys.path:
        sys.path.append(_p)

import numpy as np
import ml_dtypes

import concourse.bass as bass
import concourse.tile as tile
from concourse import bacc, mybir
from concourse.bass import IndirectOffsetOnAxis
from concourse.bass_utils import run_bass_kernel_spmd

F32 = mybir.dt.float32
BF16 = mybir.dt.bfloat16
I32 = mybir.dt.int32
AF = mybir.ActivationFunctionType
ALU = mybir.AluOpType

B, T, V, E, H, TAGS = 128, 512, 50000, 256, 512, 64
NC = 8
NPAIR = NC // 2
BL = B // NPAIR          # 32 sequences per core (one direction each)
G = 4 * H
GROUPS = [[c, c + NPAIR] for c in range(NPAIR)]
NTOK = BL * T            # 16384 tokens per core, time-major: tok = t*32+b

# gate order i,f,o,g: psum quadrant q holds gate q, sigmoid covers 0:96
_GATE_PERM = np.concatenate([
    np.arange(0, H), np.arange(H, 2 * H), np.arange(3 * H, 4 * H),
    np.arange(2 * H, 3 * H)])


def _build(nc, Tn=T, Bl=BL):
    nchunk = NTOK // 128     # 128 token chunks (4 steps each)
    ncls = nchunk // 2       # 64 classifier chunks per core
    KE = E // 128            # 2
    KH = H // 128            # 4
    KH2 = 2 * H // 128       # 8

    emb = nc.dram_tensor("emb", [V, E], F32, kind="ExternalInput")
    xg_idx = nc.dram_tensor("xg_idx", [128, nchunk], I32, kind="ExternalInput")
    mask = nc.dram_tensor("mask", [Bl, Tn], F32, kind="ExternalInput")
    ident = nc.dram_tensor("ident", [32, 32], BF16, kind="ExternalInput")
    id128 = nc.dram_tensor("id128", [128, 128], BF16, kind="ExternalInput")
    flagF = nc.dram_tensor("flagF", [128, 1], F32, kind="ExternalInput")
    flagB = nc.dram_tensor("flagB", [128, 1], F32, kind="ExternalInput")

    wih, whh, biasd = {}, {}, {}
    for l, din in (("l1", E), ("l2", 2 * H)):
        wih[l] = nc.dram_tensor(f"wihT_{l}", [din, G], BF16, kind="ExternalInput")
        whh[l] = nc.dram_tensor(f"whhT_{l}", [H, G], BF16, kind="ExternalInput")
        biasd[l] = nc.dram_tensor(f"bias_{l}", [128, G], F32, kind="ExternalInput")
    wcls = nc.dram_tensor("wclsT", [2 * H, TAGS], BF16, kind="ExternalInput")
    bcls = nc.dram_tensor("bcls", [TAGS, 1], F32, kind="ExternalInput")

    gx = {l: nc.dram_tensor(f"gx_{l}", [NTOK, G], BF16) for l in ("l1", "l2")}
    hT = {l: [nc.dram_tensor(f"hT_{l}_{j}", [128, Tn * 32], BF16)
              for j in range(4)] for l in ("l1", "l2")}
    hTp = {l: [nc.dram_tensor(f"hTp_{l}_{j}", [2 * 128, Tn * 32], BF16)
               for j in range(4)] for l in ("l1", "l2")}
    logitsT = nc.dram_tensor("logitsT", [TAGS, NTOK // 2], F32,
                             kind="ExternalOutput")

    # Every core writes its hT in OWN-scan order (uniform). In the
    # exchanged buffer slot0 = fwd core's hT (columns = fwd time) and
    # slot1 = bwd core's (columns = bwd scan time = T-1-fwd). A core
    # reading x2 at its own scan step t needs the OWN slot natural and
    # the CROSS slot time-reversed; which slot is which depends on the
    # core, so both variants are loaded (cheap contiguous DMA) and
    # selected with a host 0/1 flag on the DVE (SPMD-uniform program).

    with tile.TileContext(nc) as tc:
        with tc.tile_pool(name="const", bufs=1) as cpool:
            def load_const(nm, shape, dt, src_ap):
                t = cpool.tile(shape, dt, name=nm, tag=nm)
                nc.gpsimd.dma_start(t[:], src_ap)
                return t

            xg_sb = load_const("xg_sb", [128, nchunk], I32, xg_idx[:])
            mask_sb = load_const("mask_sb", [Bl, Tn], F32, mask[:])
            id_sb = load_const("id_sb", [32, 32], BF16, ident[:])
            id128_sb = load_const("id128_sb", [128, 128], BF16, id128[:])
            bcls_sb = load_const("bcls_sb", [TAGS, 1], F32, bcls[:])
            fF_sb = load_const("fF_sb", [128, 1], F32, flagF[:])
            fB_sb = load_const("fB_sb", [128, 1], F32, flagB[:])
            bias_sb = {l: load_const(f"bias_sb_{l}", [128, G], F32, biasd[l][:])
                       for l in ("l1", "l2")}

            _proj1(nc, tc, nchunk, KE, wih["l1"], bias_sb["l1"], gx["l1"],
                   emb, xg_sb, id128_sb)
            _scan(nc, tc, Tn, Bl, KH, whh["l1"], gx["l1"], hT["l1"],
                  mask_sb, id_sb)
            for j in range(4):
                nc.gpsimd.collective_compute(
                    "AllGather", ALU.bypass, GROUPS,
                    ins=[hT["l1"][j][:]], outs=[hTp["l1"][j][:]])
            _proj2(nc, tc, nchunk, Tn, wih["l2"], bias_sb["l2"], gx["l2"],
                   hTp["l1"], fF_sb, fB_sb)
            _scan(nc, tc, Tn, Bl, KH, whh["l2"], gx["l2"], hT["l2"],
                  mask_sb, id_sb)
            for j in range(4):
                nc.gpsimd.collective_compute(
                    "AllGather", ALU.bypass, GROUPS,
                    ins=[hT["l2"][j][:]], outs=[hTp["l2"][j][:]])
            _classifier(nc, tc, ncls, Tn, wcls, bcls_sb, hTp["l2"], logitsT,
                        KH2, fF_sb, fB_sb)
    return nc


def _proj1(nc, tc, nchunk, KE, wih_d, bias_t, gx_d, emb, xg_sb, id128_sb):
    """gx1 = emb[x] @ W_ih1^T + b, time-major chunks of 128 tokens.
    Embedding rows gathered (fp32), cast, transposed on the PE."""
    with tc.tile_pool(name="pw", bufs=1) as wpool, \
         tc.tile_pool(name="pg", bufs=3) as gpool, \
         tc.tile_pool(name="pxps", bufs=2, space="PSUM") as xpspool, \
         tc.tile_pool(name="pps", bufs=4, space="PSUM") as ppool, \
         tc.tile_pool(name="pout", bufs=3) as opool:
        wsb = wpool.tile([128, KE, G], BF16, tag="w", name="wih1_sb")
        for k in range(KE):
            nc.gpsimd.dma_start(wsb[:, k, :], wih_d[128 * k:128 * (k + 1), :])
        for s in range(nchunk):
            e32 = gpool.tile([128, E], F32, tag="e32")
            nc.gpsimd.indirect_dma_start(
                out=e32[:], out_offset=None, in_=emb[:],
                in_offset=IndirectOffsetOnAxis(ap=xg_sb[:, s:s + 1], axis=0))
            e16 = gpool.tile([128, E], BF16, tag="e16")
            nc.vector.tensor_copy(e16[:], e32[:])
            xps = xpspool.tile([128, E], F32, tag="xps", name="xps")
            for kk in range(KE):
                nc.tensor.matmul(xps[:, 128 * kk:128 * (kk + 1)],
                                 e16[:, 128 * kk:128 * (kk + 1)], id128_sb[:],
                                 start=True, stop=True, skip_group_check=True)
            xT = gpool.tile([128, E], BF16, tag="xT")
            nc.scalar.activation(xT[:], xps[:], AF.Copy)
            gout = opool.tile([128, G], BF16, tag="gout")
            for n in range(4):
                ps = ppool.tile([128, 512], F32, tag="ps", name="pps")
                for kk in range(KE):
                    nc.tensor.matmul(
                        ps[:], xT[:, 128 * kk:128 * (kk + 1)],
                        wsb[:, kk, 512 * n:512 * (n + 1)],
                        start=(kk == 0), stop=(kk == KE - 1),
                        skip_group_check=True)
                nc.vector.tensor_tensor(
                    out=gout[:, 512 * n:512 * (n + 1)], in0=ps[:],
                    in1=bias_t[:, 512 * n:512 * (n + 1)], op=ALU.add)
            nc.gpsimd.dma_start(gx_d[128 * s:128 * (s + 1), :], gout[:])


def _load_x2(nc, xpool, hv, s, Tn, fF, fB, tag):
    """Load x2^T [128, 8, 4, 32] for chunk s (this core's scan steps
    4s..4s+4): slot d chunks k at rows 128d.. Both the natural and the
    time-reversed variant of each slot are loaded (contiguous DMA) and
    blended with the core's 0/1 flags: own slot natural, cross reversed.
    hv dims: [d, p, k, t, c32] (k before t so AP dim order matches dst)."""
    # tiles are t-major [128, d, t4, k4, c32] so each (slot, variant) is
    # ONE DMA: src AP dims (t, k, c) with strides (128, 32, 1); the
    # reversed variant just walks t backwards. Matmul lhsT for K-chunk kk
    # is the strided slice [:, d, :, kk&3, :].
    xn = xpool.tile([128, 2, 4, 4, 32], BF16, tag=tag + "n")
    xr = xpool.tile([128, 2, 4, 4, 32], BF16, tag=tag + "r")
    qn, tn0 = s // 32, (4 * s) % 128
    hi = Tn - 1 - 4 * s
    qr, hl = hi // 128, hi % 128
    rsl = slice(hl, None, -1) if hl - 4 < 0 else slice(hl, hl - 4, -1)
    for d in range(2):
        nc.gpsimd.dma_start(xn[:, d, :, :, :],
                            hv[qn][d, :, tn0:tn0 + 4, :, :])
        nc.gpsimd.dma_start(xr[:, d, :, :, :], hv[qr][d, :, rsl, :, :])
    # select per K-chunk so xT ends up K-MAJOR [p, d, k, t, c] with each
    # chunk's (t, c) contiguous - a legal one-free-dim matmul stationary.
    # slot0 (fwd dir): natural on fwd cores, reversed on bwd cores;
    # slot1 (bwd dir): the opposite.
    xT = xpool.tile([128, 2, 4, 4, 32], BF16, tag=tag)
    a = xpool.tile([128, 4, 32], BF16, tag=tag + "a")
    b = xpool.tile([128, 4, 32], BF16, tag=tag + "b")
    for d in range(2):
        fn, fr = (fF, fB) if d == 0 else (fB, fF)
        for k in range(4):
            nc.vector.tensor_scalar_mul(a[:], xn[:, d, :, k, :], fn[:, 0:1])
            nc.vector.tensor_scalar_mul(b[:], xr[:, d, :, k, :], fr[:, 0:1])
            nc.vector.tensor_tensor(out=xT[:, d, k, :, :], in0=a[:], in1=b[:],
                                    op=ALU.add)
    return xT


def _proj2(nc, tc, nchunk, Tn, wih_d, bias_t, gx_d, hTp_d, fF, fB):
    """gx2 = [out_f | out_b] @ W_ih2^T + b in this core's own scan order."""
    hv = [q.ap().rearrange("(d p) (t k c) -> d p t k c", d=2, k=4, c=32)
          for q in hTp_d]
    KD = 8
    with tc.tile_pool(name="qw", bufs=1) as wpool, \
         tc.tile_pool(name="qx", bufs=3) as xpool, \
         tc.tile_pool(name="qps", bufs=4, space="PSUM") as ppool, \
         tc.tile_pool(name="qout", bufs=3) as opool:
        wsb = wpool.tile([128, KD, G], BF16, tag="w", name="wih2_sb")
        for k in range(KD):
            nc.gpsimd.dma_start(wsb[:, k, :], wih_d[128 * k:128 * (k + 1), :])
        for s in range(nchunk):
            xT = _load_x2(nc, xpool, hv, s, Tn, fF, fB, "xT")
            gout = opool.tile([128, G], BF16, tag="gout")
            for n in range(4):
                ps = ppool.tile([128, 512], F32, tag="ps", name="qpps")
                for kk in range(KD):
                    nc.tensor.matmul(
                        ps[:], xT[:, kk // 4, kk % 4, :, :],
                        wsb[:, kk, 512 * n:512 * (n + 1)],
                        start=(kk == 0), stop=(kk == KD - 1),
                        skip_group_check=True)
                nc.vector.tensor_tensor(
                    out=gout[:, 512 * n:512 * (n + 1)], in0=ps[:],
                    in1=bias_t[:, 512 * n:512 * (n + 1)], op=ALU.add)
            nc.gpsimd.dma_start(gx_d[128 * s:128 * (s + 1), :], gout[:])


def _classifier(nc, tc, ncls, Tn, wcls_d, bcls_sb, hTp_d, logitsT, KH2,
                fF, fB):
    """logits for this core's half of the pair's tokens: chunks s=0..63 of
    its OWN scan time (host un-reverses bwd cores)."""
    hv = [q.ap().rearrange("(d p) (t k c) -> d p t k c", d=2, k=4, c=32)
          for q in hTp_d]
    with tc.tile_pool(name="cw", bufs=1) as wpool, \
         tc.tile_pool(name="cx", bufs=3) as xpool, \
         tc.tile_pool(name="cps", bufs=4, space="PSUM") as ppool, \
         tc.tile_pool(name="cout", bufs=3) as opool:
        wsb = wpool.tile([128, KH2, TAGS], BF16, tag="w", name="wcls_sb")
        for k in range(KH2):
            nc.gpsimd.dma_start(wsb[:, k, :], wcls_d[128 * k:128 * (k + 1), :])
        for s in range(ncls):
            o2T = _load_x2(nc, xpool, hv, s, Tn, fF, fB, "o2T")
            ps = ppool.tile([TAGS, 128], F32, tag="ps", name="cpps")
            for kk in range(KH2):
                nc.tensor.matmul(ps[:], wsb[:, kk, :],
                                 o2T[:, kk // 4, kk % 4, :, :],
                                 start=(kk == 0), stop=(kk == KH2 - 1),
                                 skip_group_check=True)
            lg = opool.tile([TAGS, 128], F32, tag="lg")
            nc.scalar.activation(lg[:], ps[:], AF.Identity,
                                 bias=bcls_sb[:, 0:1])
            nc.gpsimd.dma_start(logitsT[:, 128 * s:128 * (s + 1)], lg[:])


def _scan(nc, tc, Tn, Bl, KH, whh_d, gx_d, hTout_d, mask_sb, id_sb):
    """Single-direction scan, M=32, col-tiled quadrant psum layout.
    Gate masks (i,f,o multiplied by mask[:,t]) implement pad-packed
    semantics; the transposed state hTn is DMA'd per step straight into
    hTout (this core's scan order)."""
    TC = 8
    gxv = gx_d.ap().rearrange("(t b) d -> b t d", b=Bl)
    hTv = [q.ap().rearrange("p (t c) -> p t c", c=128) for q in hTout_d]
    with tc.tile_pool(name="sw", bufs=1) as wpool, \
         tc.tile_pool(name="sgx", bufs=3) as gxpool, \
         tc.tile_pool(name="sst", bufs=1) as stpool, \
         tc.tile_pool(name="sps", bufs=2, space="PSUM") as pspool, \
         tc.tile_pool(name="stps", bufs=2, space="PSUM") as tpspool, \
         tc.tile_pool(name="swk", bufs=3) as wkpool, \
         tc.tile_pool(name="shT", bufs=3) as htpool, \
         tc.tile_pool(name="srng", bufs=2) as rpool:
        wsb = wpool.tile([128, KH, G], BF16, tag="whh", name="whh_sb")
        for k in range(KH):
            nc.gpsimd.dma_start(wsb[:, k, :], whh_d[128 * k:128 * (k + 1), :])
        hT = [htpool.tile([128, KH * Bl], BF16, tag="hT", name="hT0")]
        nc.vector.memset(hT[0][:], 0.0)
        c_st = stpool.tile([Bl, H], F32, tag="c", name="c_st")
        nc.vector.memset(c_st[:], 0.0)
        gxc = {}
        gps = [None]
        ring = [None]
        nwin = (Tn + TC - 1) // TC

        def load_gx(w):
            tl = gxpool.tile([Bl, TC, G], BF16, tag="gx", name="gxc")
            nc.gpsimd.dma_start(tl[:], gxv[:, w * TC:(w + 1) * TC, :])
            gxc[w] = tl
            gxc.pop(w - 2, None)

        def inject(tt):
            gps[0] = pspool.tile([128, H], F32, tag="ps", name="gps")
            gxt = gxc[tt // TC]
            j = tt % TC
            for q in range(4):
                nc.tensor.matmul(
                    gps[0][32 * q:32 * (q + 1), :], id_sb[:],
                    gxt[:, j, 512 * q:512 * (q + 1)],
                    start=True, stop=False, tile_position=(0, 32 * q),
                    skip_group_check=True)

        load_gx(0)
        if nwin > 1:
            load_gx(1)
        inject(0)
        for t in range(Tn):
            gc = gps[0]
            for k in range(KH):
                for q in range(4):
                    nc.tensor.matmul(
                        gc[32 * q:32 * (q + 1), :],
                        hT[0][:, Bl * k:Bl * (k + 1)],
                        wsb[:, k, 512 * q:512 * (q + 1)],
                        start=False, stop=(k == KH - 1),
                        tile_position=(0, 32 * q), skip_group_check=True)
            gact = wkpool.tile([128, H], BF16, tag="gact", name="gact")
            nc.scalar.activation(gact[0:96, :], gc[0:96, :], AF.Sigmoid)
            # i masked in place (base 0), f/o realigned+masked while tanh(g)
            # runs on ScalarE; mask=0 freezes h=c=0 (pad-packed semantics)
            gi0 = wkpool.tile([Bl, H], BF16, tag="gi0", name="gi0")
            nc.vector.tensor_scalar_mul(gi0[:], gact[0:32, :],
                                        mask_sb[:, t:t + 1])
            gf0 = wkpool.tile([Bl, H], BF16, tag="gf0", name="gf0")
            nc.vector.tensor_copy(gf0[:], gact[32:64, :])
            gf0m = wkpool.tile([Bl, H], BF16, tag="gf0m", name="gf0m")
            nc.vector.tensor_scalar_mul(gf0m[:], gf0[:], mask_sb[:, t:t + 1])
            gg0 = wkpool.tile([Bl, H], BF16, tag="gg0", name="gg0")
            nc.scalar.activation(gg0[:], gc[96:128, :], AF.Tanh)
            t1 = wkpool.tile([Bl, H], F32, tag="t1", name="t1")
            nc.vector.tensor_tensor(out=t1[:], in0=gf0m[:], in1=c_st[:],
                                    op=ALU.mult)
            t2 = wkpool.tile([Bl, H], BF16, tag="t2", name="t2")
            nc.vector.tensor_tensor(out=t2[:], in0=gi0[:], in1=gg0[:],
                                    op=ALU.mult)
            nc.vector.tensor_tensor(out=c_st[:], in0=t1[:], in1=t2[:],
                                    op=ALU.add)
            tch = wkpool.tile([Bl, H], BF16, tag="tch", name="tch")
            nc.scalar.activation(tch[:], c_st[:], AF.Tanh)
            go0 = wkpool.tile([Bl, H], BF16, tag="go0", name="go0")
            nc.vector.tensor_copy(go0[:], gact[64:96, :])
            go0m = wkpool.tile([Bl, H], BF16, tag="go0m", name="go0m")
            nc.vector.tensor_scalar_mul(go0m[:], go0[:], mask_sb[:, t:t + 1])
            h16 = wkpool.tile([Bl, H], BF16, tag="h16", name="h16")
            nc.vector.tensor_tensor(out=h16[:], in0=go0m[:], in1=tch[:],
                                    op=ALU.mult)
            hT_ps = tpspool.tile([128, KH * Bl], F32, tag="tps", name="hT_ps")
            if t + 1 < Tn:
                if (t + 1) % TC == 0 and (t + 1) // TC + 1 < nwin:
                    load_gx((t + 1) // TC + 1)
                inject(t + 1)
            hTn = htpool.tile([128, KH * Bl], BF16, tag="hT", name="hTn")
            for k in range(KH):
                nc.tensor.matmul(hT_ps[:, Bl * k:Bl * (k + 1)],
                                 h16[:, 128 * k:128 * (k + 1)], id_sb[:],
                                 start=True, stop=True, skip_group_check=True)
            nc.scalar.activation(hTn[:], hT_ps[:], AF.Copy)
            hT[0] = hTn
            # ring-batch the hT writes: one [128, 4, 128] DMA per 4 steps
            if t % 4 == 0:
                ring[0] = rpool.tile([128, 4, 128], BF16, tag="rng",
                                     name="ring")
            nc.vector.tensor_copy(ring[0][:, t % 4, :], hTn[:])
            if (t + 1) % 4 == 0:
                jq, t0 = (t - 3) // 128, (t - 3) % 128
                nc.gpsimd.dma_start(hTv[jq][:, t0:t0 + 4, :], ring[0][:])


def _prep_inputs(inputs, Tn=T, Bl=BL):
    x = np.asarray(inputs["x"]).astype(np.int32)
    lengths = np.asarray(inputs["lengths"]).astype(np.int32)
    emb = np.asarray(inputs["emb"], dtype=np.float32)
    bf = ml_dtypes.bfloat16

    wt = {}
    for s in ("f1", "b1", "f2", "b2"):
        w_ih = np.asarray(inputs[f"W_ih_{s}"], np.float32)[_GATE_PERM]
        w_hh = np.asarray(inputs[f"W_hh_{s}"], np.float32)[_GATE_PERM]
        b = np.asarray(inputs[f"b_{s}"], np.float32)[_GATE_PERM]
        wt[f"wihT_{s}"] = np.ascontiguousarray(w_ih.T).astype(bf)
        wt[f"whhT_{s}"] = np.ascontiguousarray(w_hh.T).astype(bf)
        wt[f"bias_{s}"] = np.tile(b.reshape(1, G), (128, 1))
    com = {"emb": emb, "ident": np.eye(32, dtype=bf),
           "id128": np.eye(128, dtype=bf),
           "wclsT": np.ascontiguousarray(
               np.asarray(inputs["W_cls"], np.float32).T).astype(bf),
           "bcls": np.asarray(inputs["b_cls"], np.float32).reshape(TAGS, 1)}

    def chunked_timemajor(xscan):
        # v[tok] = xscan[b, t] with tok = t*32 + b  ->  idx[p, s] = v[128s+p]
        v = np.ascontiguousarray(xscan.T).reshape(-1)   # [t, b] flat
        return np.ascontiguousarray(v.reshape(-1, 128).T).astype(np.int32)

    ts = np.arange(Tn)[None, :]
    in_maps = [None] * NC
    for p in range(NPAIR):
        xs = x[Bl * p:Bl * (p + 1), :Tn]
        ls = np.minimum(lengths[Bl * p:Bl * (p + 1)], Tn)[:, None]
        for half, core in ((0, p), (1, p + NPAIR)):
            if half == 0:   # forward
                xscan = xs
                m = (ts < ls).astype(np.float32)
                sfx = ("f1", "f2")
            else:           # backward: global time flip + tail mask
                xscan = xs[:, ::-1]
                m = (ts >= Tn - ls).astype(np.float32)
                sfx = ("b1", "b2")
            fl = 1.0 if half == 0 else 0.0
            im = {"xg_idx": chunked_timemajor(xscan), "mask": m,
                  "flagF": np.full((128, 1), fl, np.float32),
                  "flagB": np.full((128, 1), 1.0 - fl, np.float32),
                  "wihT_l1": wt[f"wihT_{sfx[0]}"],
                  "whhT_l1": wt[f"whhT_{sfx[0]}"],
                  "bias_l1": wt[f"bias_{sfx[0]}"],
                  "wihT_l2": wt[f"wihT_{sfx[1]}"],
                  "whhT_l2": wt[f"whhT_{sfx[1]}"],
                  "bias_l2": wt[f"bias_{sfx[1]}"]}
            im.update(com)
            in_maps[core] = im
    return in_maps


_CACHED = {}


def kernel(**inputs) -> np.ndarray:
    if "nc" not in _CACHED:
        nc = bacc.Bacc("TRN2", target_bir_lowering=False, debug=False,
                       num_devices=NC)
        _build(nc)
        nc.compile()
        _CACHED["nc"] = nc
    nc = _CACHED["nc"]
    in_maps = _prep_inputs(inputs)
    res = run_bass_kernel_spmd(nc, in_maps, core_ids=list(range(NC)),
                               trace=False)
    out = np.empty((B, T, TAGS), np.float32)
    half_T = T // 2
    for p in range(NPAIR):
        for half, core in ((0, p), (1, p + NPAIR)):
            lt = res.results[core]["logitsT"]          # [TAGS, 8192]
            seq = lt.T.reshape(half_T, BL, TAGS)       # [t_scan, b, TAGS]
            seq = np.transpose(seq, (1, 0, 2))         # [b, t_scan, TAGS]
            if half == 0:   # fwd core: scan time = fwd time 0..256
                out[BL * p:BL * (p + 1), 0:half_T] = seq
            else:           # bwd core: scan steps 0..256 = fwd time 511..256
                out[BL * p:BL * (p + 1), half_T:T] = seq[:, ::-1]
    return out.astype(np.float32)
